# revision 1
# baseline (speedup 1.0000x reference)
"""Trainium2 Bass kernel for DenseCapsule dynamic routing (3 iterations).

Problem: x[128,2048,8] f32, weight[16,2048,16,8] f32 -> out[128,16,16] f32.
  x_hat = einsum('oide,bie->boid', W, x); 3 routing iterations
  (softmax over o, c-weighted i-sum, squash, agreement update).

Strategy (8 NeuronCores, shard in_num_caps I=2048 -> 256 per core):
  x_hat is never materialized. Per iteration, everything factors through W:
    iter1: c uniform -> s1 = (1/16) * [x @ W]   (one fp32r matmul chain)
    u = v . W (PE), l = sum_e x*u (DVE/GPSIMD), softmax (ACT/DVE),
    xc = c*x (DVE), s = xc @ W (PE).
  Cross-core: AllReduce of partial s ([128,16,16] f32) after iters 1 and 2;
  the final iteration's partial s is returned per-core and the host does the
  gather-sum + final squash (that is the "unshard" step).

Layout conventions per core (SBUF partition dim first):
  i_local = ih*128 + il  (ih in {0,1}, il = partition 0..127)
  o = 4*h + g            (g in 0..3 selects a 32-partition group, h in 0..3)
  d padded to 32 rows (dd) for the u-matmul stationary operand (K=32 rule).
"""

import sys

for _p in ("/opt/trn_rl_repo", "/root/.axon_site/_ro/trn_rl_repo"):
    if _p not in sys.path:
        sys.path.insert(0, _p)

import numpy as np
import ml_dtypes

import concourse.bass as bass
import concourse.bacc as bacc
import concourse.mybir as mybir
import concourse.tile as tile
from concourse.bass_utils import run_bass_kernel_spmd

F32 = mybir.dt.float32
F32R = mybir.dt.float32r
BF16 = mybir.dt.bfloat16
NPBF16 = ml_dtypes.bfloat16

N_CORES = 8
B = 128          # batch
I_FULL = 2048    # in caps
IC = 256         # in caps per core
IL = 128         # partition dim of i
IH = IC // IL    # 2
E = 8            # in cap dim
O = 16           # out caps
D = 16           # out cap dim
EPS = 1e-8

_CACHE = {}


def _emit_squash(nc, pool, sfull, vpad, tag):
    """squash on [(b)=128, (h,g,d)=256] f32 layout; writes v into vpad
    ([(b), (h,g,dd=32)=512] f32, pad rows stay zero)."""
    sq = pool.tile([B, O * D], F32, tag="sq")
    nc.scalar.square(sq[:, :], sfull[:, :])
    nrm2 = pool.tile([B, O], F32, tag="nrm2")
    # reduce innermost d (16) of (o=16, d=16)
    nc.vector.reduce_sum(
        nrm2[:, :],
        sq[:, :].rearrange("p (o d) -> p o d", d=D),
        axis=mybir.AxisListType.X,
    )
    q = pool.tile([B, O], F32, tag="q")
    nc.scalar.sqrt(q[:, :], nrm2[:, :])
    t1 = pool.tile([B, O], F32, tag="t1")
    nc.gpsimd.tensor_scalar_add(t1[:, :], nrm2[:, :], 1.0)
    t2 = pool.tile([B, O], F32, tag="t2")
    nc.vector.tensor_scalar_add(t2[:, :], q[:, :], EPS)
    den = pool.tile([B, O], F32, tag="den")
    nc.vector.tensor_mul(den[:, :], t1[:, :], t2[:, :])
    rden = pool.tile([B, O], F32, tag="rden")
    nc.vector.reciprocal(rden[:, :], den[:, :])
    scale = pool.tile([B, O], F32, tag="scale")
    nc.gpsimd.tensor_mul(scale[:, :], nrm2[:, :], rden[:, :])
    # v = s * scale (broadcast over d) into vpad[(b), (h, g, dd<16)]
    s_v = sfull[:, :].rearrange("p (h g d) -> p h g d", h=4, g=4)
    scale_v = scale[:, :].rearrange("p (h g) -> p h g", h=4).broadcast_to(
        (B, 4, 4, D)
    )
    vslice = vpad[:, :].rearrange("p (h g dd) -> p h g dd", h=4, g=4)[:, :, :, 0:D]
    nc.vector.tensor_tensor(vslice, s_v, scale_v, op=mybir.AluOpType.mult)


def _emit_transpose_v(nc, psum_pool, pool, vpad, vT, ident, tag):
    """vpad [(b), (h, g, dd)=512] f32 -> vT [(g,dd)=128, (h,b)=512] bf16
    via 4 PE transposes (one per h) + ACT evacuations."""
    for h in range(4):
        tp = psum_pool.tile([128, B], F32, tag="ps")
        in_slice = vpad[:, h * 128:(h + 1) * 128]
        nc.tensor.transpose(tp[:, :], in_slice, ident[:, :])
        nc.scalar.copy(vT[:, h * B:(h + 1) * B], tp[:, :])


def _emit_iteration_ul(nc, tc, pools, vT, l_buf, delta_buf, wdt, xbf, itr):
    """u = v.W (PE) -> evac (ACT) -> xu = x*u (DVE) -> e-reduction rounds
    (DVE/GPSIMD) -> l (or delta for iter 3)."""
    pool, psum_pool, seq = pools
    for o in range(O):
        h, g = o // 4, o % 4
        u_ps = psum_pool.tile([IL, IH * E * B], F32, tag="ps")
        for ih in range(IH):
            for e in range(E):
                # lhsT: wdt[(32g..32g+32), (h, ih, e, il)] -> [32, 128]
                lhsT = wdt[:, :].rearrange(
                    "p (h ih e il) -> p h ih e il", h=4, ih=IH, e=E
                )[32 * g:32 * (g + 1), h, ih, e, :]
                rhs = vT[32 * g:32 * (g + 1), h * B:(h + 1) * B]
                nc.tensor.matmul(
                    u_ps[:, (ih * E + e) * B:(ih * E + e + 1) * B], lhsT, rhs,
                    start=True, stop=True, tile_position=(32 * g, 0),
                )
        u_sb = pool.tile([IL, IH * E * B], BF16, tag="u_sb")
        nc.scalar.copy(u_sb[:, :], u_ps[:, :])
        # xu = x * u   (x layout (ih,e,b) matches u layout (ih,e,b))
        xu = pool.tile([IL, IH * E * B], BF16, tag="xu")
        nc.vector.tensor_tensor(
            xu[:, :], xbf[:, :], u_sb[:, :], op=mybir.AluOpType.mult,
        )
        # e-reduction rounds within each ih block: 8 -> 4 -> 2 -> 1
        xu4 = xu[:, :].rearrange("p (ih half eb) -> p ih half eb",
                                 ih=IH, half=2)
        r1 = pool.tile([IL, IH * 4 * B], BF16, tag="r1")
        r1v = r1[:, :].rearrange("p (ih eb) -> p ih eb", ih=IH)
        nc.vector.tensor_tensor(r1v, xu4[:, :, 0], xu4[:, :, 1],
                                op=mybir.AluOpType.add)
        r1h = r1[:, :].rearrange("p (ih half eb) -> p ih half eb",
                                 ih=IH, half=2)
        r2 = pool.tile([IL, IH * 2 * B], BF16, tag="r2")
        r2v = r2[:, :].rearrange("p (ih eb) -> p ih eb", ih=IH)
        nc.vector.tensor_tensor(r2v, r1h[:, :, 0], r1h[:, :, 1],
                                op=mybir.AluOpType.add)
        r2h = r2[:, :].rearrange("p (ih half b) -> p ih half b",
                                 ih=IH, half=2)
        dst_buf = l_buf if itr == 2 else delta_buf
        dst = dst_buf[:, :].rearrange(
            "p (o ih b) -> p o ih b", o=O, ih=IH
        )[:, o]
        nc.vector.tensor_tensor(dst, r2h[:, :, 0], r2h[:, :, 1],
                                op=mybir.AluOpType.add)
    if itr == 3:
        nc.vector.tensor_add(l_buf[:, :], l_buf[:, :], delta_buf[:, :])


def _emit_softmax_xc_s(nc, tc, pools, l_buf, xbf, wbf, s_ps, itr):
    """exp (ACT), Z (DVE), 1/Z, xprime = x/Z, then per-o xc = exp*xprime and
    the 16 accumulating s-matmuls into s_ps [(b), (h,g,d)=256]."""
    pool, psum_pool, seq = pools
    # exp split by o-half: the first half and its Z-subtree overlap the tail
    # of the ul chain (o 0-7 logits land ~8us before o 8-15).
    exp_buf = seq.tile([IL, O * IH * B], BF16, tag="exp")
    HALF = 8 * IH * B
    nc.scalar.activation(
        exp_buf[:, 0:HALF], l_buf[:, 0:HALF],
        mybir.ActivationFunctionType.Exp)
    za1 = seq.tile([IL, 4 * IH * B], F32, tag="za1")
    nc.vector.tensor_add(za1[:, :], exp_buf[:, 0:HALF // 2],
                         exp_buf[:, HALF // 2:HALF])
    za2 = seq.tile([IL, 2 * IH * B], F32, tag="za2")
    nc.vector.tensor_add(za2[:, :], za1[:, 0:2 * IH * B],
                         za1[:, 2 * IH * B:4 * IH * B])
    za3 = seq.tile([IL, IH * B], F32, tag="za3")
    nc.vector.tensor_add(za3[:, :], za2[:, 0:IH * B],
                         za2[:, IH * B:2 * IH * B])
    nc.scalar.activation(
        exp_buf[:, HALF:2 * HALF], l_buf[:, HALF:2 * HALF],
        mybir.ActivationFunctionType.Exp)
    zb1 = seq.tile([IL, 4 * IH * B], F32, tag="zb1")
    nc.vector.tensor_add(zb1[:, :], exp_buf[:, HALF:HALF + HALF // 2],
                         exp_buf[:, HALF + HALF // 2:2 * HALF])
    zb2 = seq.tile([IL, 2 * IH * B], F32, tag="zb2")
    nc.vector.tensor_add(zb2[:, :], zb1[:, 0:2 * IH * B],
                         zb1[:, 2 * IH * B:4 * IH * B])
    zb3 = seq.tile([IL, IH * B], F32, tag="zb3")
    nc.vector.tensor_add(zb3[:, :], zb2[:, 0:IH * B],
                         zb2[:, IH * B:2 * IH * B])
    zbuf = seq.tile([IL, IH * B], F32, tag="z")
    nc.vector.tensor_add(zbuf[:, :], za3[:, :], zb3[:, :])
    rz = seq.tile([IL, IH * B], F32, tag="rz")
    nc.vector.reciprocal(rz[:, :], zbuf[:, :])
    rzbf = seq.tile([IL, IH * B], BF16, tag="rzbf")
    nc.vector.tensor_copy(rzbf[:, :], rz[:, :])
    xp = seq.tile([IL, IH * E * B], BF16, tag="xp")
    nc.vector.tensor_tensor(
        xp[:, :].rearrange("p (ih e b) -> p ih e b", ih=IH, e=E),
        xbf[:, :].rearrange("p (ih e b) -> p ih e b", ih=IH, e=E),
        rzbf[:, :].rearrange("p (ih b) -> p ih b", ih=IH)
        .unsqueeze(2).broadcast_to((IL, IH, E, B)),
        op=mybir.AluOpType.mult,
    )
    for o in range(O):
        h, g = o // 4, o % 4
        xc = pool.tile([IL, IH * E * B], BF16, tag="xc")
        nc.vector.tensor_tensor(
            xc[:, :].rearrange("p (ih e b) -> p ih e b", ih=IH, e=E),
            exp_buf[:, :].rearrange("p (o ih b) -> p o ih b", o=O, ih=IH)[:, o]
            .unsqueeze(2).broadcast_to((IL, IH, E, B)),
            xp[:, :].rearrange("p (ih e b) -> p ih e b", ih=IH, e=E),
            op=mybir.AluOpType.mult,
        )
        n_k = IH * E
        kt = 0
        for ih in range(IH):
            for e in range(E):
                lhsT = xc[:, :].rearrange(
                    "p (ih e b) -> p ih e b", ih=IH, e=E
                )[:, ih, e, :]
                rhs = wbf[:, :].rearrange(
                    "p (ih e o d) -> p ih e o d", ih=IH, e=E, o=O
                )[:, ih, e, o, :]
                nc.tensor.matmul(
                    s_ps[:, o * D:(o + 1) * D], lhsT, rhs,
                    start=(kt == 0), stop=(kt == n_k - 1),
                )
                kt += 1


def build():
    nc = bacc.Bacc("TRN2", target_bir_lowering=False, debug=False,
                   enable_asserts=True, num_devices=N_CORES)

    # per-core inputs (host pre-arranged; see kernel())
    xbf_d = nc.dram_tensor("xbf", [IL, IH * E * B], BF16,
                           kind="ExternalInput").ap()
    wbf_d = nc.dram_tensor("wbf", [IL, IH * E * O * D], BF16,
                           kind="ExternalInput").ap()
    wdt_d = nc.dram_tensor("wdt", [128, 4 * IH * E * IL], BF16,
                           kind="ExternalInput").ap()
    ident_d = nc.dram_tensor("ident", [128, 128], F32,
                             kind="ExternalInput").ap()
    sp_out = nc.dram_tensor("sp", [B, O * D], F32, kind="ExternalOutput").ap()

    cc1_in = nc.dram_tensor("cc1_in", [B, O * D], F32)
    cc1_out = nc.dram_tensor("cc1_out", [B, O * D], F32, addr_space="Shared")
    cc2_in = nc.dram_tensor("cc2_in", [B, O * D], F32)
    cc2_out = nc.dram_tensor("cc2_out", [B, O * D], F32, addr_space="Shared")

    rg = [list(range(N_CORES))]

    with tile.TileContext(nc) as tc:
        with (
            tc.tile_pool(name="const", bufs=1) as cpool,
            tc.tile_pool(name="work", bufs=4) as pool,
            tc.tile_pool(name="psum", bufs=2, space="PSUM") as psum_pool,
            tc.tile_pool(name="seq", bufs=1) as seq_pool,
            
        ):
            # ---- load inputs ----
            wbf = cpool.tile([IL, IH * E * O * D], BF16)
            nc.sync.dma_start(out=wbf[:, :], in_=wbf_d)
            wdt = cpool.tile([128, 4 * IH * E * IL], BF16)
            nc.sync.dma_start(out=wdt[:, :], in_=wdt_d)
            ident = cpool.tile([128, 128], F32)
            nc.sync.dma_start(out=ident[:, :], in_=ident_d)

            xbf = cpool.tile([IL, IH * E * B], BF16)
            nc.sync.dma_start(out=xbf[:, :], in_=xbf_d)

            l_buf = cpool.tile([IL, O * IH * B], BF16)
            delta_buf = cpool.tile([IL, O * IH * B], BF16)
            vpad = cpool.tile([B, 4 * 4 * 32], F32)
            nc.vector.memset(vpad[:, :], 0.0)
            vT = cpool.tile([128, 4 * B], BF16)

            pools = (pool, psum_pool, seq_pool)

            # ---- iteration 1: uniform c -> s1 = (1/16) x @ W ----
            _sid_s1, _ = nc.enter_named_scope("s1", False)
            s_ps1 = psum_pool.tile([B, O * D], F32, tag="ps")
            kt = 0
            for ih in range(IH):
                for e in range(E):
                    lhsT = xbf[:, :].rearrange(
                        "p (ih e b) -> p ih e b", ih=IH, e=E
                    )[:, ih, e, :]
                    rhs = wbf[:, :].rearrange(
                        "p (ih e od) -> p ih e od", ih=IH, e=E
                    )[:, ih, e, :]
                    nc.tensor.matmul(
                        s_ps1[:, :], lhsT, rhs,
                        start=(kt == 0), stop=(kt == IH * E - 1),
                    )
                    kt += 1
            s_sb1 = cpool.tile([B, O * D], F32)
            nc.scalar.mul(s_sb1[:, :], s_ps1[:, :], 1.0 / O)
            nc.sync.dma_start(out=cc1_in[:], in_=s_sb1[:, :])
            nc.leave_named_scope("s1", _sid_s1, False); _sid_ar1, _ = nc.enter_named_scope("ar1", False)
            nc.gpsimd.collective_compute(
                "AllReduce", mybir.AluOpType.add, replica_groups=rg,
                ins=[cc1_in[:]], outs=[cc1_out[:]],
            )
            sfull1 = cpool.tile([B, O * D], F32)
            nc.sync.dma_start(out=sfull1[:, :], in_=cc1_out[:])
            nc.leave_named_scope("ar1", _sid_ar1, False); _sid_squash1, _ = nc.enter_named_scope("squash1", False)
            _emit_squash(nc, cpool, sfull1, vpad, tag="1")
            _emit_transpose_v(nc, psum_pool, cpool, vpad, vT, ident, tag="1")
            nc.leave_named_scope("squash1", _sid_squash1, False)

            # ---- iteration 2 ----
            _sid_ul2, _ = nc.enter_named_scope("ul2", False)
            _emit_iteration_ul(nc, tc, pools, vT, l_buf, delta_buf, wdt, xbf, 2)
            nc.leave_named_scope("ul2", _sid_ul2, False); _sid_xcs2, _ = nc.enter_named_scope("xcs2", False)
            s_ps2 = psum_pool.tile([B, O * D], F32, tag="ps")
            _emit_softmax_xc_s(nc, tc, pools, l_buf, xbf, wbf, s_ps2, 2)
            s_sb2 = cpool.tile([B, O * D], F32)
            nc.scalar.copy(s_sb2[:, :], s_ps2[:, :])
            nc.sync.dma_start(out=cc2_in[:], in_=s_sb2[:, :])
            nc.leave_named_scope("xcs2", _sid_xcs2, False); _sid_ar2, _ = nc.enter_named_scope("ar2", False)
            nc.gpsimd.collective_compute(
                "AllReduce", mybir.AluOpType.add, replica_groups=rg,
                ins=[cc2_in[:]], outs=[cc2_out[:]],
            )
            sfull2 = cpool.tile([B, O * D], F32)
            nc.sync.dma_start(out=sfull2[:, :], in_=cc2_out[:])
            nc.leave_named_scope("ar2", _sid_ar2, False); _sid_squash2, _ = nc.enter_named_scope("squash2", False)
            _emit_squash(nc, cpool, sfull2, vpad, tag="2")
            _emit_transpose_v(nc, psum_pool, cpool, vpad, vT, ident, tag="2")
            nc.leave_named_scope("squash2", _sid_squash2, False)

            # ---- iteration 3 (final: partial s3 out, host finishes) ----
            _sid_ul3, _ = nc.enter_named_scope("ul3", False)
            _emit_iteration_ul(nc, tc, pools, vT, l_buf, delta_buf, wdt, xbf, 3)
            nc.leave_named_scope("ul3", _sid_ul3, False); _sid_xcs3, _ = nc.enter_named_scope("xcs3", False)
            s_ps3 = psum_pool.tile([B, O * D], F32, tag="ps")
            _emit_softmax_xc_s(nc, tc, pools, l_buf, xbf, wbf, s_ps3, 3)
            nc.leave_named_scope("xcs3", _sid_xcs3, False)
            sp_sb = cpool.tile([B, O * D], F32)
            nc.scalar.copy(sp_sb[:, :], s_ps3[:, :])
            nc.sync.dma_start(out=sp_out, in_=sp_sb[:, :])

    nc.compile()
    return nc


def _host_prep(x, weight):
    """Build the per-core input maps (free host-side rearrangement)."""
    in_maps = []
    ident = np.eye(128, dtype=np.float32)
    for c in range(N_CORES):
        x_c = x[:, c * IC:(c + 1) * IC, :]          # [B, 256, E]
        w_c = weight[:, c * IC:(c + 1) * IC, :, :]  # [O, 256, D, E]

        # xt [il, (ih, e, b)]
        xr = x_c.reshape(B, IH, IL, E)              # b, ih, il, e
        xt = np.ascontiguousarray(
            xr.transpose(2, 1, 3, 0)                # il, ih, e, b
        ).reshape(IL, IH * E * B)

        # w [il, (ih, e, h, g, d)] with o = 4h + g
        wr = w_c.reshape(4, 4, IH, IL, D, E)        # h, g, ih, il, d, e
        w_f = np.ascontiguousarray(
            wr.transpose(3, 2, 5, 0, 1, 4)          # il, ih, e, h, g, d
        ).reshape(IL, IH * E * O * D)

        # wdt [(g, dd=32), (h, ih, e, il)] (dd >= 16 zero)
        wdtv = np.zeros((4, 32, 4, IH, E, IL), dtype=np.float32)
        wdtv[:, :D] = wr.transpose(1, 4, 0, 2, 5, 3)  # g, d, h, ih, e, il
        wdt = wdtv.reshape(128, 4 * IH * E * IL)

        in_maps.append({
            "xbf": xt.astype(NPBF16),
            "wbf": w_f.astype(NPBF16),
            "wdt": wdt.astype(NPBF16),
            "ident": ident,
        })
    return in_maps


def _host_finish(partials):
    """Sum the 8 per-core partial s3 tensors, final squash (the unshard)."""
    s = np.zeros((B, O * D), dtype=np.float64)
    for p in partials:
        s += p.astype(np.float64)
    s = s.reshape(B, O, D)
    n2 = (s * s).sum(axis=-1, keepdims=True)
    n = np.sqrt(n2)
    v = (n2 / (1.0 + n2) / (n + EPS)) * s
    return v.astype(np.float32)


def kernel(x, weight, _trace=False):
    x = np.asarray(x, dtype=np.float32)
    weight = np.asarray(weight, dtype=np.float32)
    if "nc" not in _CACHE:
        _CACHE["nc"] = build()
    nc = _CACHE["nc"]
    in_maps = _host_prep(x, weight)
    res = run_bass_kernel_spmd(
        nc, in_maps, core_ids=list(range(N_CORES)), trace=_trace
    )
    out = _host_finish([res.results[c]["sp"] for c in range(N_CORES)])
    if _trace:
        _CACHE["last_result"] = res
    return out


if __name__ == "__main__":
    rng = np.random.default_rng(0)
    x = rng.standard_normal((B, I_FULL, E)).astype(np.float32)
    w = (0.01 * rng.standard_normal((O, I_FULL, D, E))).astype(np.float32)
    out = kernel(x, w)
    print("out", out.shape, out.dtype, np.abs(out).max())

